# revision 37
# baseline (speedup 1.0000x reference)
"""Bass/Tile Trainium2 kernel for nn_Attention_9929964388721.

Module: 4-head spatial attention over [b=4, c=256, 64, 64] images.
  qkv = w_qkv @ x  (1x1 conv), split q/k/v with heads=4, dim_head=32,
  q,k l2-normalized over dim_head, sim = 10 * q^T k  (n=4096 tokens),
  attn = softmax(sim), out = attn @ v, y = w_out @ out + b_out.

Sharding (8 cores): core c handles batch b = c//2 and heads {2*(c%2), 2*(c%2)+1}.
Each core computes its partial y contribution [256, 4096]; the host sums the
two partials per batch and adds the bias.

Per-core pipeline (unit = one (batch, head) pair; 2 units/core), matmuls in
fp16 (fp32 PSUM accumulate):
  P1:  x over 3 DMA queues; half-granular u0 chains (grouped ssq proj ->
       ln/exp rsqrt halves on ACT -> rr_d halves -> 4-way-split partition
       broadcasts) so the P2-gating qs/kg muls launch early, with the vT
       projection (x-stationary, 4-jt batched copies) and qrep chunks
       filling PE dependency holes; then u1's P1 in plain order.
       rsqrt as exp(-0.5*ln(x)) on ACT, one table load for the whole kernel.
  P2:  quad-batched emission keeps same-kind matmuls back-to-back on the
       PE (a per-j-tile sim/PV alternation serializes the array and exposes
       every LDWEIGHTS -- measured 2x slower):
         simQ(q): 4 sim matmuls j=4q..4q+3 (K=32, row groups 0..3) into two
                  2-bank PSUM pair tiles -> 4-way concurrent.
         PVQ(q-3): 4 PV matmuls col-packed 2-way: even j -> av[0:33] at col
                  base 0, odd j -> av[64:97] at tile_position (0,64); the
                  ones column in vt gives per-strip softmax rowsums.
         exp on pairs [128,2,512] (one instr per 2 j-tiles): ACT table exp /
                  DVE Schraudolph int16 affine, 9/7 split per chunk.
       PSUM: 3x st pair tiles (6 banks) + av + y = 8.
  P3:  per-chunk epilogue via chunk-boundary hooks (+1 av copy PSUM->SBUF,
       rowsum rows DMA-gathered to [128,8], tiny DVE add+recip, rcp->DRAM;
       +2 rcp partition-broadcast to strips; +3 oT strip muls on Pool into a
       persistent [96,N] tile and y projection with a strip-folded W2
       [96,128] whose rows 32..63 are zero, then fp16 store; drain chunks
       compress to +1/+2 with DVE muls and 4-way split stores).
"""

import sys

sys.path.insert(0, "/opt/trn_rl_repo")

from collections import defaultdict, deque
from contextlib import ExitStack

import types

import numpy as np

import bass_rust as _bass_rust
import concourse.hw_specs as hw_specs
import concourse.mybir as mybir
import concourse.tile as tile
from concourse import bacc
from concourse.bass_utils import run_bass_kernel_spmd

HEADS = 4
DIM_HEAD = 32
B, C, H, W = 4, 256, 64, 64
N = H * W                  # 4096 tokens
HIDDEN = HEADS * DIM_HEAD  # 128
NCORES = 8
UNITS = 2                  # (batch, head) pairs per core
CHUNK = 512                # i-chunk width
NCHUNK = N // CHUNK        # 8
JT = N // 128              # 32 j-tiles of 128
NQUAD = JT // 4            # 8 quads per (unit, chunk)
F32 = mybir.dt.float32
F16 = mybir.dt.float16
I16 = mybir.dt.int16
LN10 = float(np.log(10.0))
# fp16 Schraudolph fast-exp constants: exp(x) ~ bitcast_f16(i16(A*x + B)),
# |rel err| <= ~3%; valid for x in [-14, 10.4] (t stays in [0, 32767])
A_SCH = 1024.0 * float(np.log2(np.e))
B_SCH = 1024.0 * (15.0 - 0.0434609)

# pair indices (of 16 per chunk) whose exp runs on DVE (rest on ACT);
# early u0 chunks carry u1's deferred projection copies on ACT, so they
# shift one more pair to the DVE
DVE_PAIRS = frozenset({2, 4, 7, 9, 11, 12, 14})
DVE_PAIRS_EARLY = frozenset({1, 3, 5, 7, 9, 11, 12, 14})

PV_LOOKAHEAD = 3  # quads between simQ emission and its PVQ

_ACT_TABLE = "natural_log_exp_and_others"  # holds exp, ln, square, copy


def _single_table_act_loads(self):
    """Pin every activation to one table (it contains all funcs we use), so
    the whole kernel needs exactly one ACT_TABLE_LOAD instead of bouncing."""
    tables = [(name, funcs if name == _ACT_TABLE else set())
              for name, funcs in
              hw_specs.get_activation_tables(self.m.arch).items()]
    _bass_rust.insert_act_table_loads(self, tables)


def _build():
    nc = bacc.Bacc("TRN2", target_bir_lowering=False, debug=False,
                   num_devices=NCORES)
    nc.insert_act_table_loads = types.MethodType(_single_table_act_loads, nc)

    # ---- DRAM I/O (host-prepped layouts; each loads as ONE dma) ----
    x_in = nc.dram_tensor("x_in", [C, N], F16, kind="ExternalInput").ap()
    wqT = nc.dram_tensor("wqT", [128, 2, UNITS, 128], F16,
                         kind="ExternalInput").ap()
    wkT = nc.dram_tensor("wkT", [128, 2, UNITS, DIM_HEAD], F16,
                         kind="ExternalInput").ap()
    wvT = nc.dram_tensor("wvT", [128, 2, 2 * DIM_HEAD], F16,
                         kind="ExternalInput").ap()
    w2T = nc.dram_tensor("w2T", [96, UNITS, 2, 128], F16,
                         kind="ExternalInput").ap()
    ones4 = nc.dram_tensor("ones4", [128, 4], F16, kind="ExternalInput").ap()
    y_out = nc.dram_tensor("y_out", [C, N], F16, kind="ExternalOutput").ap()

    with ExitStack() as top:
        tc = top.enter_context(tile.TileContext(nc))
        persist = top.enter_context(tc.tile_pool(name="persist", bufs=1))
        p12 = top.enter_context(ExitStack())
        chains = p12.enter_context(tc.tile_pool(name="chains", bufs=1))
        dram = top.enter_context(tc.tile_pool(name="dram", bufs=1, space="DRAM"))

        qs = [persist.tile([128, N], F16, name=f"qs{u}", tag=f"qs{u}")
              for u in range(UNITS)]
        kg = [persist.tile([128, N // 4], F16, name=f"kg{u}", tag=f"kg{u}")
              for u in range(UNITS)]
        kg_raw = [persist.tile([128, N // 4], F32, name=f"kg_raw{u}",
                               tag=f"kg_raw{u}") for u in range(UNITS)]
        vt = [persist.tile([128, JT, DIM_HEAD + 1], F16, name=f"vt{u}",
                           tag=f"vt{u}") for u in range(UNITS)]
        w_2 = persist.tile([96, UNITS, 2, 128], F16, name="w_2", tag="w_2")
        nc.gpsimd.dma_start(out=w_2[:, :, :, :], in_=w2T)

        rr_d = dram.tile([UNITS, 2, N], F32, name="rr_d", tag="rr_d")
        rcp_d = dram.tile([UNITS, NCHUNK, CHUNK], F32, name="rcp_d",
                          tag="rcp_d")

        # ---- P1b(u): broadcast + normalize muls, piecewise closures ----
        # rqb/rkb quarters share tags across units: lifetimes don't overlap
        # (u0's P1b completes before u1's pieces fire in early P2).

        def p1b_pieces(u):
            pieces = []

            dma4 = [nc.sync, nc.scalar, nc.gpsimd, nc.sync]

            def qs_piece(hh):
                def fn():
                    hs = slice(hh * (N // 4), (hh + 1) * (N // 4))
                    rqb = chains.tile([128, N // 4], F32, name="rqb",
                                      tag=f"rqb{hh % 2}")
                    # 512KB broadcast: split by partition quarter across 4
                    # DMA queues (a single queue takes ~17us for this)
                    for a in range(4):
                        dma4[a].dma_start(
                            out=rqb[32 * a:32 * a + 32, :],
                            in_=rr_d[u, 0, hs].partition_broadcast(32))
                    # Pool for the mul: DVE carries the exp stream in P2
                    meng = nc.vector if u == 0 and hh == 0 else nc.gpsimd
                    meng.tensor_mul(qs[u][:, hs], q_rep[u][:, hs],
                                    rqb[:, :])
                return fn

            def kg_piece(hh):
                def fn():
                    rkv = rr_d[u, 1, :].rearrange("(c r jj) -> r c jj", r=4,
                                                  jj=128)
                    rkb = chains.tile([128, N // 8], F32, name="rkb",
                                      tag=f"rkb{hh}")
                    for r in range(4):
                        dma4[r].dma_start(
                            out=rkb[32 * r:32 * r + 32, :].rearrange(
                                "p (c jj) -> p c jj", jj=128),
                            in_=rkv[r, 4 * hh:4 * hh + 4, :]
                            .partition_broadcast(32))
                    hs = slice(hh * (N // 8), (hh + 1) * (N // 8))
                    # u1's kg muls run at the last pre-u1 chunk where Pool
                    # is backlogged: DVE is safer there
                    meng = nc.vector if (u == 0 and hh == 0) or u == 1 \
                        else nc.gpsimd
                    meng.tensor_mul(kg[u][:, hs], kg_raw[u][:, hs],
                                    rkb[:, :])
                return fn

            for hh in range(4):
                pieces.append(qs_piece(hh))
            for hh in range(2):
                pieces.append(kg_piece(hh))
            return pieces

        # Only the pieces P2(u0) chunk 0 needs run before P2: qs quarter 0
        # (i 0..1024) and kg half 0 (j-tiles 0..15). The rest hook into the
        # first P2 chunks, just ahead of their first use.

        # =========================== P1a: projections ======================
        wpool = p12.enter_context(tc.tile_pool(name="wpool", bufs=1))
        with ExitStack() as p1:
            sc = p1.enter_context(tc.tile_pool(name="p1scratch", bufs=2))
            pq = p1.enter_context(tc.tile_pool(name="pq", bufs=2, space="PSUM"))
            pssq = p1.enter_context(tc.tile_pool(name="pssq", bufs=2,
                                                 space="PSUM"))
            pk = p1.enter_context(tc.tile_pool(name="pk", bufs=2, space="PSUM"))
            pv = p1.enter_context(tc.tile_pool(name="pv", bufs=2, space="PSUM"))

            w_q = wpool.tile([128, 2, UNITS, 128], F16, name="w_q", tag="w_q")
            w_k = wpool.tile([128, 2, UNITS, DIM_HEAD], F16, name="w_k",
                             tag="w_k")
            w_v = wpool.tile([128, 2, 2 * DIM_HEAD], F16, name="w_v", tag="w_v")
            o4 = wpool.tile([128, 4], F16, name="o4", tag="o4")

            # x spread across FOUR DMA queues so grouped_proj's x chunks all
            # land early; weights ride the vector/gpsimd queues (small).
            x_sb = wpool.tile([128, 2, N], F16, name="x_sb", tag="x_sb")
            x_view = x_in.rearrange("(kt p) n -> p kt n", p=128)
            nc.sync.dma_start(out=w_v[:, :, :], in_=wvT)
            # x chunk 0 split across both HWDGE queues: its two halves
            # transfer in parallel, so the first vT matmul starts sooner
            nc.scalar.dma_start(out=x_sb[:, :, 0:CHUNK // 2],
                                in_=x_view[:, :, 0:CHUNK // 2])
            nc.sync.dma_start(out=x_sb[:, :, CHUNK // 2:CHUNK],
                              in_=x_view[:, :, CHUNK // 2:CHUNK])
            nc.gpsimd.dma_start(out=w_q[:, :, :, :], in_=wqT)
            nc.gpsimd.dma_start(out=w_k[:, :, :, :], in_=wkT)
            nc.gpsimd.dma_start(out=o4[:, :], in_=ones4)
            xq = [nc.sync, nc.scalar, nc.gpsimd]
            for ch in range(1, NCHUNK):
                xq[ch % 3].dma_start(
                    out=x_sb[:, :, ch * CHUNK:(ch + 1) * CHUNK],
                    in_=x_view[:, :, ch * CHUNK:(ch + 1) * CHUNK])

            q_rep = [chains.tile([128, N], F32, name=f"q_rep{u}",
                                 tag=f"q_rep{u}") for u in range(UNITS)]
            ln10_t = chains.tile([4, 1], F32, name="ln10_t", tag="ln10_t")
            nc.vector.memset(ln10_t[:, :], LN10)

            sstq = [chains.tile([4, NCHUNK, 128], F32, name=f"sstq{u}",
                             tag=f"sstq{u}") for u in range(UNITS)]
            sstk = [chains.tile([4, NCHUNK, 128], F32, name=f"sstk{u}",
                             tag=f"sstk{u}") for u in range(UNITS)]

            def qrep_proj_chunks(u, c0, c1):
                # q replicated projection (for the sim matmuls)
                for ch in range(c0, c1):
                    ps = pq.tile([128, CHUNK], F32, name="psq", tag="psq")
                    for kt in range(2):
                        nc.tensor.matmul(
                            ps[:, :],
                            w_q[:, kt, u, :],
                            x_sb[:, kt, ch * CHUNK:(ch + 1) * CHUNK],
                            start=(kt == 0), stop=(kt == 1))
                    dst = q_rep[u][:, ch * CHUNK:(ch + 1) * CHUNK]
                    if ch % 2 == 0:
                        nc.scalar.copy(dst, ps[:, :])
                    else:
                        nc.vector.tensor_copy(dst, ps[:, :])

            def grouped_proj(u, w_t, wd, h):
                ps = pk.tile([128, CHUNK], F32, name="psk", tag="psk")
                for r in range(4):
                    for kt in range(2):
                        xv = x_sb[:, kt, :].rearrange(
                            "p (blk cc jj) -> p blk cc jj", cc=4, jj=128)
                        nc.tensor.matmul(
                            ps[32 * r:32 * r + 32, :],
                            w_t[:, kt, u, 0:wd],
                            xv[:, 4 * h:4 * h + 4, r, :],
                            start=(kt == 0), stop=(kt == 1),
                            tile_position=(0, 32 * r))
                return ps

            for u in range(UNITS):
                nc.vector.memset(vt[u][:, :, DIM_HEAD:DIM_HEAD + 1], 1.0)

            def vt_proj(jt4_lo, jt4_hi):
                for jt4 in range(jt4_lo, jt4_hi):
                    ps = pv.tile([128, 4, 2 * DIM_HEAD], F32, name="psv",
                                 tag="psv")
                    for s in range(4):
                        jt = 4 * jt4 + s
                        for kt in range(2):
                            nc.tensor.matmul(
                                ps[:, s, :],
                                x_sb[:, kt, jt * 128:(jt + 1) * 128],
                                w_v[:, kt, :],
                                start=(kt == 0), stop=(kt == 1))
                    for u in range(UNITS):
                        nc.vector.tensor_copy(
                            vt[u][:, 4 * jt4:4 * jt4 + 4, 0:DIM_HEAD],
                            ps[:, :, u * DIM_HEAD:(u + 1) * DIM_HEAD])

            def ssq_q(u, h):
                ps = grouped_proj(u, w_q, DIM_HEAD, h)
                sq = sc.tile([128, CHUNK], F16, name="sq", tag="sq")
                nc.scalar.activation(sq[:, :], ps[:, :],
                                     mybir.ActivationFunctionType.Square)
                ps2 = pssq.tile([4, CHUNK], F32, name="psssq", tag="psssq")
                nc.tensor.matmul(ps2[:, :], o4[:, :], sq[:, :],
                                 start=True, stop=True)
                nc.vector.tensor_copy(
                    sstq[u][:, 4 * h:4 * h + 4, :],
                    ps2[:, :].rearrange("r (cc jj) -> r cc jj", jj=128))

            def ssq_k(u, h):
                ps = grouped_proj(u, w_k, DIM_HEAD, h)
                nc.vector.tensor_copy(
                    kg_raw[u][:, h * CHUNK:(h + 1) * CHUNK], ps[:, :])
                sq = sc.tile([128, CHUNK], F16, name="sq", tag="sq")
                kr = kg_raw[u][:, h * CHUNK:(h + 1) * CHUNK]
                nc.vector.tensor_mul(sq[:, :], kr, kr)
                ps2 = pssq.tile([4, CHUNK], F32, name="psssq", tag="psssq")
                nc.tensor.matmul(ps2[:, :], o4[:, :], sq[:, :],
                                 start=True, stop=True)
                nc.vector.tensor_copy(
                    sstk[u][:, 4 * h:4 * h + 4, :],
                    ps2[:, :].rearrange("r (cc jj) -> r cc jj", jj=128))

            def lnexp(sst, u, h, qbias):
                cc = slice(4 * h, 4 * h + 4)
                nc.scalar.activation(sst[u][:, cc, :], sst[u][:, cc, :],
                                     mybir.ActivationFunctionType.Ln)
                nc.scalar.activation(sst[u][:, cc, :], sst[u][:, cc, :],
                                     mybir.ActivationFunctionType.Exp,
                                     bias=ln10_t[:, :] if qbias else 0.0,
                                     scale=-0.5)
                eng = nc.gpsimd if qbias else nc.sync
                which = 0 if qbias else 1
                eng.dma_start(
                    out=rr_d[u, which, 2048 * h:2048 * h + 2048].rearrange(
                        "(c r jj) -> r c jj", r=4, jj=128),
                    in_=sst[u][:, cc, :])

            # --- u0: half-granular chains so the gating broadcast launches
            # as early as possible; PE dependency holes are filled with the
            # vT projection and qrep chunks ---
            p1b0 = p1b_pieces(0)
            ssq_q(0, 0)
            lnexp(sstq, 0, 0, True)
            qrep_proj_chunks(0, 0, 2)
            p1b0[0]()            # qs quarter 0 bcast+mul (gates P2)
            vt_proj(0, 4)
            ssq_q(0, 1)
            lnexp(sstq, 0, 1, True)
            ssq_k(0, 0)
            lnexp(sstk, 0, 0, False)
            p1b0[4]()            # kg half 0
            vt_proj(4, 8)
            ssq_k(0, 1)
            lnexp(sstk, 0, 1, False)
            qrep_proj_chunks(0, 2, 8)

            # --- u1: plain order ---
            for h in range(2):
                ssq_q(1, h)
            for h in range(2):
                ssq_k(1, h)
            lnexp(sstq, 1, 0, True)
            lnexp(sstq, 1, 1, True)
            lnexp(sstk, 1, 0, False)
            lnexp(sstk, 1, 1, False)
            qrep_proj_chunks(1, 0, 8)

        # ======================= P2 + pipelined epilogue ===================
        sc3 = p12.enter_context(tc.tile_pool(name="p3scratch", bufs=4))

        # per-unit oT strip tiles [96, N]: rows 0..31 = strip E (even j),
        # rows 64..95 = strip O (odd j), rows 32..63 contracted with zero
        # weights in W2 but must hold finite fp16 (NaN*0 = NaN on the PE)
        ot2 = [persist.tile([96, N], F16, name=f"ot2{u}", tag=f"ot2{u}")
               for u in range(UNITS)]
        for u in range(UNITS):
            nc.gpsimd.memset(ot2[u][32:64, :], 0.0)

        rsb_tiles = {}
        avsb_tiles = {}
        av_tiles = {}

        def st_avcopy(u, ch):
            # av bank -> SBUF (fp32), rowsum rows DMA-gathered to [128, 8],
            # rcp = 1/(rsE+rsO) as a tiny [128,4] DVE op, bounce via DRAM
            def fn():
                avsb = sc3.tile([128, CHUNK], F32, name="avsb", tag="avsb")
                avsb_tiles[(u, ch)] = avsb
                av = av_tiles.pop((u, ch))
                nc.vector.tensor_copy(avsb[:, :], av[:, :])
                rsq = sc3.tile([128, 8], F32, name="rsq", tag="rsq")
                nc.gpsimd.dma_start(out=rsq[:, 0:4],
                                    in_=avsb[DIM_HEAD:DIM_HEAD + 1, :])
                nc.sync.dma_start(out=rsq[:, 4:8],
                                  in_=avsb[64 + DIM_HEAD:64 + DIM_HEAD + 1, :])
                rcp = sc3.tile([128, 4], F32, name="rcp", tag="rcp")
                nc.vector.tensor_add(rcp[:, :], rsq[:, 0:4], rsq[:, 4:8])
                nc.vector.reciprocal(rcp[:, :], rcp[:, :])
                nc.gpsimd.dma_start(
                    out=rcp_d[u, ch, :].rearrange("(p f) -> p f", p=128),
                    in_=rcp[:, :])
            return fn

        def st_bcast(u, ch):
            def fn():
                rsb = sc3.tile([96, CHUNK], F32, name="rsb", tag="rsb")
                rsb_tiles[(u, ch)] = rsb
                nc.sync.dma_start(
                    out=rsb[0:32, :],
                    in_=rcp_d[u, ch, :].partition_broadcast(32))
                nc.gpsimd.dma_start(
                    out=rsb[64:96, :],
                    in_=rcp_d[u, ch, :].partition_broadcast(32))
            return fn

        def st_ot(u, ch):
            # normalize both strips into the persistent oT strip tile
            # (on Pool: SBUF-only inputs, and the DVE carries the exp stream;
            # drain chunks use the by-then-idle DVE, which is ~3x faster)
            def fn():
                cs = slice(ch * CHUNK, (ch + 1) * CHUNK)
                avsb = avsb_tiles.pop((u, ch))
                rsb = rsb_tiles.pop((u, ch))
                meng = (nc.vector if (u == 1 and ch >= NCHUNK - 2)
                        else nc.gpsimd)
                meng.tensor_mul(ot2[u][0:DIM_HEAD, cs],
                                avsb[0:DIM_HEAD, :], rsb[0:32, :])
                meng.tensor_mul(ot2[u][64:64 + DIM_HEAD, cs],
                                avsb[64:64 + DIM_HEAD, :], rsb[64:96, :])
            return fn

        def st_y(ch):
            # strip-folded y projection over both units + fp16 store.
            # Chunks whose y lands in the post-loop drain use the freed st
            # PSUM tags so the two m-halves don't serialize on one bank.
            drain = ch >= NCHUNK - 3

            def fn():
                cs = slice(ch * CHUNK, (ch + 1) * CHUNK)
                for m in range(2):
                    if drain:
                        ps = pst.tile([128, CHUNK], F32, name="psy",
                                      tag=f"st{(2 * ch + m) % 3}")
                    else:
                        ps = py.tile([128, CHUNK], F32, name="psy", tag="psy")
                    for uu in range(UNITS):
                        nc.tensor.matmul(
                            ps[:, :],
                            w_2[:, uu, m, :],
                            ot2[uu][:, cs],
                            start=(uu == 0), stop=(uu == 1))
                    ysb = sc3.tile([128, CHUNK], F16, name="ysb", tag="ysb",
                                   bufs=2)
                    if (ch + m) % 2 == 0:
                        nc.scalar.copy(ysb[:, :], ps[:, :])
                    else:
                        nc.vector.tensor_copy(ysb[:, :], ps[:, :])
                    if drain:
                        # split the 128KB store across queues to shorten
                        # the post-loop tail
                        engs = [nc.sync, nc.scalar, nc.gpsimd, nc.scalar]
                        for half in range(2):
                            pr = slice(64 * half, 64 * half + 64)
                            engs[(2 * ch + 2 * m + half) % 4].dma_start(
                                out=y_out[m * 128 + 64 * half:
                                          m * 128 + 64 * half + 64, cs],
                                in_=ysb[pr, :])
                    else:
                        eng = [nc.sync, nc.gpsimd][(2 * ch + m) % 2]
                        eng.dma_start(out=y_out[m * 128:(m + 1) * 128, cs],
                                      in_=ysb[:, :])
            return fn

        with ExitStack() as p2:
            pst = p2.enter_context(tc.tile_pool(name="pst", bufs=1,
                                                space="PSUM"))
            pav = p2.enter_context(tc.tile_pool(name="pav", bufs=1,
                                                space="PSUM"))
            py = p2.enter_context(tc.tile_pool(name="py", bufs=1,
                                               space="PSUM"))
            ptp = p2.enter_context(tc.tile_pool(name="ptp", bufs=3))

            hooks = defaultdict(list)
            hooks[0].extend([p1b0[5], p1b0[1]])   # kg half 1, qs quarter 1
            hooks[1].append(p1b0[2])              # qs quarter 2
            hooks[2].append(p1b0[3])              # qs quarter 3
            for i, fn in enumerate(p1b_pieces(1)):
                hooks[4 + i // 2].append(fn)
            for u in range(UNITS):
                for ch in range(NCHUNK):
                    s = u * NCHUNK + ch
                    if u == 1 and ch >= NCHUNK - 3:
                        # drain-phase chunks: compress the chain so the
                        # post-loop tail is short (deps order the DMAs)
                        hooks[s + 1].extend([st_avcopy(u, ch),
                                             st_bcast(u, ch)])
                        hooks[s + 2].extend([st_ot(u, ch), st_y(ch)])
                    else:
                        hooks[s + 1].append(st_avcopy(u, ch))
                        hooks[s + 2].append(st_bcast(u, ch))
                        hooks[s + 3].append(st_ot(u, ch))
                        if u == 1:
                            hooks[s + 3].append(st_y(ch))

            pending = deque()

            def emit_pvq(p):
                u, st2a, pta, st2b, ptb, jt0, av, dve_a, dve_b = p
                # exps were already emitted at sim time; here only the PVs
                for (pt2, jbase) in ((pta, jt0), (ptb, jt0 + 2)):
                    for s in range(2):
                        j = jbase + s
                        strip = j % 2
                        nc.tensor.matmul(
                            av[64 * strip:64 * strip + DIM_HEAD + 1, :],
                            vt[u][:, j, :],
                            pt2[:, s, :],
                            start=(j < 2), stop=(j >= JT - 2),
                            tile_position=(0, 64 * strip))

            def emit_exp(st2, pt2, on_dve):
                if on_dve:
                    nc.vector.tensor_scalar(
                        pt2[:, :, :].bitcast(I16), st2[:, :, :],
                        A_SCH, B_SCH,
                        mybir.AluOpType.mult, mybir.AluOpType.add)
                else:
                    nc.scalar.activation(pt2[:, :, :], st2[:, :, :],
                                         mybir.ActivationFunctionType.Exp)

            for u in range(UNITS):
                for ch in range(NCHUNK):
                    gch = u * NCHUNK + ch
                    i0 = ch * CHUNK
                    chunk_hooks = deque(hooks.pop(gch, []))
                    av = pav.tile([128, CHUNK], F32, name="av", tag="av")
                    av_tiles[(u, ch)] = av
                    for q in range(NQUAD):
                        pair0 = 2 * q
                        sts, pts = [], []
                        for pp in (pair0, pair0 + 1):
                            tag = pp % 3
                            st2 = pst.tile([128, 2, CHUNK], F32,
                                           name=f"st{tag}", tag=f"st{tag}")
                            pt2 = ptp.tile([128, 2, CHUNK], F16, name="pt",
                                           tag=f"pt{pp % 3}")
                            sts.append(st2)
                            pts.append(pt2)
                        # 4 sim matmuls back-to-back (row groups 0..3)
                        for s4 in range(4):
                            j = 4 * q + s4
                            r = j % 4
                            t = j // 4
                            nc.tensor.matmul(
                                sts[s4 // 2][:, s4 % 2, :],
                                kg[u][32 * r:32 * r + 32,
                                      t * 128:(t + 1) * 128],
                                qs[u][32 * r:32 * r + 32, i0:i0 + CHUNK],
                                start=True, stop=True,
                                tile_position=(32 * r, 0))
                        # PVs for the quad a few slots back
                        if len(pending) >= PV_LOOKAHEAD:
                            emit_pvq(pending.popleft())
                        # exps for this quad
                        dve_a = pair0 % 16 in DVE_PAIRS
                        dve_b = (pair0 + 1) % 16 in DVE_PAIRS
                        emit_exp(sts[0], pts[0], dve_a)
                        emit_exp(sts[1], pts[1], dve_b)
                        pending.append((u, sts[0], pts[0], sts[1], pts[1],
                                        4 * q, av, dve_a, dve_b))
                        # spread boundary work between quads
                        if q % 2 == 1 and chunk_hooks:
                            chunk_hooks.popleft()()
                    while chunk_hooks:
                        chunk_hooks.popleft()()
            while pending:
                emit_pvq(pending.popleft())
            for g in sorted(hooks):
                for fn in hooks[g]:
                    fn()

        p12.close()

    nc.compile()
    return nc


_NC_CACHE = None


def _get_nc():
    global _NC_CACHE
    if _NC_CACHE is None:
        _NC_CACHE = _build()
    return _NC_CACHE


def _make_in_maps(x, w_qkv, w_out):
    """Build the 8 per-core input dicts from full inputs."""
    x = np.ascontiguousarray(x, dtype=np.float32)
    w_qkv = np.ascontiguousarray(w_qkv, dtype=np.float32)
    w_out = np.ascontiguousarray(w_out, dtype=np.float32)
    b, c, h, w = x.shape
    xf = x.reshape(b, c, h * w)

    ones4 = np.zeros((128, 4), np.float16)
    for r in range(4):
        ones4[32 * r:32 * r + 32, r] = 1.0

    in_maps = []
    for core in range(NCORES):
        bb = core // 2
        p = core % 2
        heads = [2 * p, 2 * p + 1]
        wq = np.stack([w_qkv[hh * DIM_HEAD:(hh + 1) * DIM_HEAD, :]
                       for hh in heads])
        wk = np.stack([w_qkv[HIDDEN + hh * DIM_HEAD:
                             HIDDEN + (hh + 1) * DIM_HEAD, :] for hh in heads])
        wv = np.stack([w_qkv[2 * HIDDEN + hh * DIM_HEAD:
                             2 * HIDDEN + (hh + 1) * DIM_HEAD, :]
                       for hh in heads])
        # wqT device tile [128p, 2kt, U, 128m]: wq[u].T replicated 4x in m
        wqTt = np.concatenate([np.transpose(wq, (0, 2, 1))] * 4, axis=2)
        wqT = np.ascontiguousarray(
            wqTt.reshape(UNITS, 2, 128, 128).transpose(2, 1, 0, 3))
        wkTt = np.transpose(wk, (0, 2, 1))  # [U, 256, 32]
        wkT = np.ascontiguousarray(
            wkTt.reshape(UNITS, 2, 128, DIM_HEAD).transpose(2, 1, 0, 3))
        wvTt = np.concatenate([wv[0].T, wv[1].T], axis=1)  # [256, 64]
        wvT = np.ascontiguousarray(
            wvTt.reshape(2, 128, 2 * DIM_HEAD).transpose(1, 0, 2))
        # w2T [96p, U, 2m, 128mm]: strip-folded output projection.
        # p in [0,32): w_out[128m+mm, head_u*32+p]; p in [64,96): same for
        # p-64; rows 32..63 zero.
        w2T = np.zeros((96, UNITS, 2, 128), np.float32)
        for u in range(UNITS):
            wo_u = w_out[:, heads[u] * DIM_HEAD:(heads[u] + 1) * DIM_HEAD]
            # wo_u [256, 32]; w2T[p, u, m, mm] = wo_u[128m+mm, p]
            for m in range(2):
                blk = wo_u[m * 128:(m + 1) * 128, :].T  # [32, 128]
                w2T[0:32, u, m, :] = blk
                w2T[64:96, u, m, :] = blk
        in_maps.append({
            "x_in": np.ascontiguousarray(xf[bb]).astype(np.float16),
            "wqT": wqT.astype(np.float16),
            "wkT": wkT.astype(np.float16),
            "wvT": wvT.astype(np.float16),
            "w2T": w2T.astype(np.float16),
            "ones4": ones4,
        })
    return in_maps


def kernel(x, w_qkv, w_out, b_out):
    nc = _get_nc()
    in_maps = _make_in_maps(x, w_qkv, w_out)
    res = run_bass_kernel_spmd(nc, in_maps, core_ids=list(range(NCORES)))
    outs = res.results
    y = np.zeros((B, C, N), np.float32)
    for bb in range(B):
        y[bb] = (outs[2 * bb]["y_out"].astype(np.float32)
                 + outs[2 * bb + 1]["y_out"].astype(np.float32))
    y += np.asarray(b_out, np.float32)[None, :, None]
    return y.reshape(B, C, H, W).astype(np.float32)
